# revision 9
# baseline (speedup 1.0000x reference)
"""BiAffine layer kernel for 8 Trainium2 NeuronCores.

Reference computation (per batch b):
  s = relu(x @ sW.T + sb)                  [L, E]
  t = relu(x @ tW.T + tb)                  [L, E]
  key = (s @ blW.T).reshape(L, E, N)
  out1[i, n, l] = sum_e key[i, e, n] * t[l, e]
  su = s @ Wu.T + f2b ; tv = t @ Wv.T      (Wu, Wv = f2W[:, :E], f2W[:, E:])
  h[i, j, :] = relu(su[i] + tv[j])
  out2[i, n, j] = sum_e h[i, j, e] * f3W[n, e] + f3b[n]
  out = out1 + out2                        [L, N, L]

Sharding: 8 cores = 2 batches x 4 blocks of 128 source positions (i).

PSUM layout: one pair-tile [128, 1024] (2 banks) holds TWO octets of 8 i's
each; within a 512-col half, rows 32k + 12s + n hold (i = 8o + 2k + s, n),
8 pad rows per 32-group carry garbage that the output DMA skips.

h production per (i, ec) chunk [128, 512] fp16, true h = relu(tv + su):
  i%8 in 0..5 -> DVE tensor_scalar (in0=tv fp16, scalar1=su[:, i] fp32,
    op0=add, op1=max 0.0) - 4x_2p mode, ~263ns issue-to-issue.
  i%8 in 6,7 -> ACT activation(Relu, bias=su[:, i]) ~714ns/op.
GpSimd cannot help: its SBUF port IS the DVE's second (perf-mode) port
with an exclusive per-instruction lock, so Pool elementwise ops stall the
DVE 4x_2p stream 1:1.  DVE 12 / ACT 4 (+flush) per octet is the wall.

key lives n-major with interleaved zeros:
  keyNZ[ec][e, 128*(12s+n) + i] = key[i, 128ec+e, n] if i%2 == s else 0
so the key matmuls write plain contiguous 128-col psum blocks and out1
needs only single-stride [128, 24] stationaries.  Both interleave casts
run on ACT (the DVE is the pacer); they land in octets 0-2 where ACT has
no flush duty yet.  out1 is deferred DEFER=4 octets so the PE is never
head-of-line blocked on the blW load.

Loads: 3 DMA queues (sync/scalar HWDGE + gpsimd SWDGE) sustain
~100-140 GB/s each when fed large contiguous per-partition runs, so
tensors ship as few wide-column DMAs (>=1.5KB/partition runs), ordered
strictly by criticality: t-inputs (x, tW) one per queue first, then the
s-chain (sW, xTi) + Wu/Wv/f3pad (packed in one small tensor), then blW
as 4 [128,1536] blocks (ec-major so ec0's key matmuls can start before
ec1's blocks arrive).

Flush: one ACT [128, 1024] copy+f3b per pair, ONE pair behind (p0 and p1
at g==2 after the deferred out1s, then p_{g-1} at each loop top), each
followed by a single contiguous [128, 1024] fp16 DMA on the sync queue.
The last pair flushes in halves: octet-14's half mid-g=7 (overlapping
octet 15), octet-15's half after the loop with its DMA split across the
scalar+gpsimd queues to shorten the tail.
"""

import sys

sys.path.insert(0, "/opt/trn_rl_repo")

import numpy as np

B, L, H, E, N = 2, 512, 768, 256, 12
EC = E // 128  # 2 e-chunks
HC = H // 128  # 6 h-chunks
IB = L // 4  # 128 i's per core
NCORES = 8
OCTS = IB // 8  # 16
PAIRS = OCTS // 2  # 8
DEFER = 4  # octets whose out1 is emitted late (key not ready yet)

# misc fp32 column layout: [sb(2) tb(2) f2b(2) f3b128(1)]
MISC_W = 7
# small fp16 tensor: [WuT (EC*E) | WvT (EC*E) | f3pad (EC*48)]
SMALL_W = EC * E * 2 + EC * 48

_cache = {}


def build_nc():
    import concourse.bass as bass
    import concourse.tile as tile
    from concourse import bacc, mybir
    from contextlib import ExitStack

    fp32 = mybir.dt.float32
    fp16 = mybir.dt.float16
    AF = mybir.ActivationFunctionType
    ALU = mybir.AluOpType

    nc = bacc.Bacc("TRN2")

    # ---- I/O (multi-chunk tensors prepacked chunk-major on host, fp16) ----
    xTm = nc.dram_tensor("xTm", [128, HC * L], fp16, kind="ExternalInput")
    tWTm = nc.dram_tensor("tWTm", [128, HC * E], fp16, kind="ExternalInput")
    xTim = nc.dram_tensor("xTim", [128, HC * IB], fp16, kind="ExternalInput")
    sWTm = nc.dram_tensor("sWTm", [128, HC * E], fp16, kind="ExternalInput")
    smallm = nc.dram_tensor("smallm", [128, SMALL_W], fp16, kind="ExternalInput")
    # blW blocks b = 2*ec + epc, each [128, 1536] = (e-in-chunk, (e_out n))
    blWm = nc.dram_tensor("blWm", [128, 4 * E * N // 2], fp16, kind="ExternalInput")
    misc = nc.dram_tensor("misc", [128, MISC_W], fp32, kind="ExternalInput")
    # raw pair-major layout: [pair, psum row (32k+12s+n, pads included), o, j];
    # host-side _gather unscrambles (and drops the 8 pad rows per 32-group)
    out = nc.dram_tensor("out", [PAIRS, 128, 2 * L], fp16, kind="ExternalOutput")

    BLK = E * N // 2  # 1536 cols per blW block

    with tile.TileContext(nc) as tc, ExitStack() as ctx:
        consts = ctx.enter_context(tc.tile_pool(name="consts", bufs=1))
        acts = ctx.enter_context(tc.tile_pool(name="acts", bufs=1))

        def tile_of(shape, name, dt=fp16):
            return consts.tile(shape, dt, name=name)

        xT_m = tile_of([128, HC * L], "xT_m")
        tWT_m = tile_of([128, HC * E], "tWT_m")
        xTi_m = tile_of([128, HC * IB], "xTi_m")
        sWT_m = tile_of([128, HC * E], "sWT_m")
        small_m = tile_of([128, SMALL_W], "small_m")
        blW_m = tile_of([128, 4 * BLK], "blW_m")
        misc_sb = tile_of([128, MISC_W], "misc_sb", dt=fp32)

        # Strict criticality order; every piece keeps >=1.5KB/partition runs
        # so each queue sustains ~120+ GB/s.  t-inputs (x, tW) lead, Wv
        # follows (gates tv), then the s-chain (sW, xTi, Wu), f3pad, blW
        # (ec0 blocks before ec1; out1 is deferred until they land).
        TL = HC * L // 3  # 1024-col x pieces (2 hc chunks each)
        TW = HC * E // 2  # 768-col tW/sW pieces (3 hc chunks each)
        nc.scalar.dma_start(out=misc_sb[:], in_=misc[:])
        nc.sync.dma_start(out=xT_m[:, :TL], in_=xTm[:, :TL])
        nc.scalar.dma_start(out=xT_m[:, TL : 2 * TL], in_=xTm[:, TL : 2 * TL])
        nc.gpsimd.dma_start(out=xT_m[:, 2 * TL :], in_=xTm[:, 2 * TL :])
        nc.sync.dma_start(out=tWT_m[:, :TW], in_=tWTm[:, :TW])
        nc.scalar.dma_start(out=tWT_m[:, TW:], in_=tWTm[:, TW:])
        nc.gpsimd.dma_start(out=xTi_m[:], in_=xTim[:])
        nc.sync.dma_start(out=small_m[:, : EC * E],
                          in_=smallm[:, : EC * E])  # Wv
        nc.scalar.dma_start(out=small_m[:, EC * E :],
                            in_=smallm[:, EC * E :])  # Wu | f3pad
        nc.sync.dma_start(out=sWT_m[:, :TW], in_=sWTm[:, :TW])
        nc.gpsimd.dma_start(out=sWT_m[:, TW:], in_=sWTm[:, TW:])
        nc.gpsimd.dma_start(out=blW_m[:, 0 * BLK : 1 * BLK],
                            in_=blWm[:, 0 * BLK : 1 * BLK])
        nc.sync.dma_start(out=blW_m[:, 1 * BLK : 2 * BLK],
                          in_=blWm[:, 1 * BLK : 2 * BLK])
        nc.gpsimd.dma_start(out=blW_m[:, 2 * BLK : 3 * BLK],
                            in_=blWm[:, 2 * BLK : 3 * BLK])
        nc.scalar.dma_start(out=blW_m[:, 3 * BLK : 4 * BLK],
                            in_=blWm[:, 3 * BLK : 4 * BLK])

        xT_sb = [xT_m[:, L * c : L * (c + 1)] for c in range(HC)]
        tWT_sb = [tWT_m[:, E * c : E * (c + 1)] for c in range(HC)]
        xTi_sb = [xTi_m[:, IB * c : IB * (c + 1)] for c in range(HC)]
        sWT_sb = [sWT_m[:, E * c : E * (c + 1)] for c in range(HC)]
        WvT_sb = [small_m[:, E * c : E * (c + 1)] for c in range(EC)]
        WuT_sb = [small_m[:, EC * E + E * c : EC * E + E * (c + 1)]
                  for c in range(EC)]
        f3pad_sb = [small_m[:, 2 * EC * E + 48 * c : 2 * EC * E + 48 * (c + 1)]
                    for c in range(EC)]
        # blW block (ec, epc) -> [128, e_out, n]
        blWT3 = {}
        for ec in range(EC):
            for epc in range(EC):
                b_ = 2 * ec + epc
                blWT3[(ec, epc)] = blW_m[:, BLK * b_ : BLK * (b_ + 1)] \
                    .rearrange("p (e n) -> p e n", n=N)
        o_ = 0
        sb_sb = misc_sb[:, o_ : o_ + 2]; o_ += 2
        tb_sb = misc_sb[:, o_ : o_ + 2]; o_ += 2
        f2b_sb = misc_sb[:, o_ : o_ + 2]; o_ += 2
        f3b_sb = misc_sb[:, o_ : o_ + 1]; o_ += 1

        # ---- persistent activations ----
        tT_sb = [acts.tile([128, L], fp16, name=f"tT{ec}") for ec in range(EC)]
        sTb_sb = [acts.tile([128, IB], fp16, name=f"sTb{ec}") for ec in range(EC)]
        tvT2c = acts.tile([128, 2 * L], fp16, name="tvT2c")  # cols 512*ec + j
        suT = acts.tile([128, 2 * IB], fp32, name="suT")  # cols IB*ec + i
        # keyNZ[ec][e, 128*(12s+n) + i] = key[i, 128ec+e, n] if i%2==s else 0
        keyNZ = [acts.tile([128, 24 * IB], fp16, name=f"keyNZ{ec}")
                 for ec in range(EC)]
        for ec in range(EC):
            # DVE is idle during the load phase
            nc.vector.memset(keyNZ[ec][:], 0.0)

        # ---- prep.  PE order: t (hc-interleaved, both ec chains), s,
        # tv0, su0, tv1, su1.  The relu-s / ident-su0 run on the (idle)
        # DVE so the ACT tail (relu-t x2 + copy-tv0) and the DVE tail
        # (relu-s + ident-su0) shorten the path to the first h op.
        with tc.tile_pool(name="prepA", bufs=3, space="PSUM") as ppA:
            ps_t = [ppA.tile([128, L], fp32, name=f"ps_t{ec}", tag="ps")
                    for ec in range(EC)]
            for hc in range(HC):
                for ec in range(EC):
                    nc.tensor.matmul(
                        ps_t[ec][:],
                        lhsT=tWT_sb[hc][:, 128 * ec : 128 * (ec + 1)],
                        rhs=xT_sb[hc],
                        start=(hc == 0),
                        stop=(hc == HC - 1),
                    )
            for ec in range(EC):
                nc.scalar.activation(tT_sb[ec][:], ps_t[ec][:], AF.Relu,
                                     bias=tb_sb[:, ec : ec + 1])

            ps_s = [ppA.tile([128, L], fp32, name=f"ps_s{ec}", tag="ps")
                    for ec in range(EC)]
            for hc in range(HC):
                for ec in range(EC):
                    nc.tensor.matmul(
                        ps_s[ec][:, :IB],
                        lhsT=sWT_sb[hc][:, 128 * ec : 128 * (ec + 1)],
                        rhs=xTi_sb[hc],
                        start=(hc == 0),
                        stop=(hc == HC - 1),
                    )
            for ec in range(EC):
                nc.vector.tensor_scalar(
                    out=sTb_sb[ec][:], in0=ps_s[ec][:, :IB],
                    scalar1=sb_sb[:, ec : ec + 1], scalar2=0.0,
                    op0=ALU.add, op1=ALU.max)

            ps_tv0 = ppA.tile([128, L], fp32, name="ps_tv0", tag="ps")
            for epc in range(EC):
                nc.tensor.matmul(
                    ps_tv0[:],
                    lhsT=WvT_sb[epc][:, 0:128],
                    rhs=tT_sb[epc][:],
                    start=(epc == 0),
                    stop=(epc == EC - 1),
                )
            nc.scalar.copy(tvT2c[:, 0:L], ps_tv0[:])

            ps_su0 = ppA.tile([128, L], fp32, name="ps_su0", tag="ps")
            for epc in range(EC):
                nc.tensor.matmul(
                    ps_su0[:, :IB],
                    lhsT=WuT_sb[epc][:, 0:128],
                    rhs=sTb_sb[epc][:],
                    start=(epc == 0),
                    stop=(epc == EC - 1),
                )
            nc.vector.tensor_scalar(
                out=suT[:, 0:IB], in0=ps_su0[:, :IB],
                scalar1=f2b_sb[:, 0:1], scalar2=None, op0=ALU.add)

            ps_tv1 = ppA.tile([128, L], fp32, name="ps_tv1", tag="ps")
            for epc in range(EC):
                nc.tensor.matmul(
                    ps_tv1[:],
                    lhsT=WvT_sb[epc][:, 128:256],
                    rhs=tT_sb[epc][:],
                    start=(epc == 0),
                    stop=(epc == EC - 1),
                )
            nc.scalar.copy(tvT2c[:, L : 2 * L], ps_tv1[:])

            ps_su1 = ppA.tile([128, L], fp32, name="ps_su1", tag="ps")
            for epc in range(EC):
                nc.tensor.matmul(
                    ps_su1[:, :IB],
                    lhsT=WuT_sb[epc][:, 128:256],
                    rhs=sTb_sb[epc][:],
                    start=(epc == 0),
                    stop=(epc == EC - 1),
                )
            nc.scalar.activation(suT[:, IB : 2 * IB], ps_su1[:, :IB],
                                 AF.Identity, bias=f2b_sb[:, 1:2])

        # ---- key: n-major psum (contiguous in-bank writes), interleave-
        # cast into the zero-padded keyNZ layout, all casts on ACT.
        ppB = ctx.enter_context(tc.tile_pool(name="prepB", bufs=1, space="PSUM"))

        def emit_key_mms(ec):
            psK = ppB.tile([128, N * IB], fp32, name="psK", tag="psK")
            for n in range(N):
                for epc in range(EC):
                    nc.tensor.matmul(
                        psK[:, IB * n : IB * (n + 1)],
                        lhsT=blWT3[(ec, epc)][:, :, n],
                        rhs=sTb_sb[epc][:],
                        start=(epc == 0),
                        stop=(epc == EC - 1),
                    )
            return psK

        def emit_key_cast(psK, ec, s):
            psK3 = psK.rearrange("p (n i) -> p n i", i=IB)
            kz3_ = keyNZ[ec].rearrange("p (m i) -> p m i", i=IB)
            src = psK3[:, :, s : IB : 2]
            dst = kz3_[:, N * s : N * (s + 1), s : IB : 2]
            nc.scalar.copy(dst, src)

        psK0 = emit_key_mms(0)

        # ---- main loop: pairs of octets; flush pipelined one pair back ----
        hp = ctx.enter_context(tc.tile_pool(name="hp", bufs=64))
        outp = ctx.enter_context(tc.tile_pool(name="outp", bufs=3))
        mp = ctx.enter_context(tc.tile_pool(name="mp", bufs=2, space="PSUM"))

        kz3 = [keyNZ[ec].rearrange("p (m i) -> p m i", i=IB) for ec in range(EC)]

        def emit_out1(ps, o, first, last=False):
            # k-inner so consecutive matmuls land on different PE col-groups
            base = 512 * (o % 2)
            for ec in range(EC):
                for s in range(2):
                    for k in range(4):
                        i = 8 * o + 2 * k + s
                        nc.tensor.matmul(
                            ps[32 * k : 32 * k + 24, base : base + 512],
                            lhsT=kz3[ec][:, :, i],
                            rhs=tT_sb[ec][:],
                            start=(first and ec == 0 and s == 0),
                            stop=(last and ec == EC - 1 and s == 1),
                            tile_position=(0, 32 * k),
                            skip_group_check=True,
                        )

        def emit_h(o):
            # ec-outer: the first ops of octet 0 need only the ec0 tv/su
            hs = {}
            for ec in range(EC):
                for p_ in range(8):
                    i = 8 * o + p_
                    ht = hp.tile([128, L], fp16, name="ht", tag="h")
                    if p_ < 6:
                        nc.vector.tensor_scalar(
                            out=ht[:], in0=tvT2c[:, L * ec : L * (ec + 1)],
                            scalar1=suT[:, IB * ec + i : IB * ec + i + 1],
                            scalar2=0.0, op0=ALU.add, op1=ALU.max)
                    else:
                        nc.scalar.activation(
                            ht[:], tvT2c[:, L * ec : L * (ec + 1)], AF.Relu,
                            bias=suT[:, IB * ec + i : IB * ec + i + 1])
                    hs[(p_, ec)] = ht
            return hs

        def emit_out2(ps, o, hs, first, last=True):
            base = 512 * (o % 2)
            for ec in range(EC):
                for p_ in (0, 2, 4, 6, 1, 3, 5, 7):
                    k, s = divmod(p_, 2)
                    nc.tensor.matmul(
                        ps[32 * k : 32 * k + 24, base : base + 512],
                        lhsT=f3pad_sb[ec][:, 24 * s : 24 * s + 24],
                        rhs=hs[(p_, ec)][:],
                        start=(first and ec == 0 and s == 0 and p_ == 2 * k),
                        stop=(last and ec == EC - 1),
                        tile_position=(0, 32 * k),
                        skip_group_check=True,
                    )

        def flush(ps_prev, g_prev):
            ob = outp.tile([128, 2 * L], fp16, name="ob")
            nc.scalar.activation(ob[:], ps_prev[:], AF.Identity, bias=f3b_sb)
            nc.sync.dma_start(out=out[g_prev], in_=ob[:])

        def flush_half(ps_prev, g_prev, half, engs):
            obh = outp.tile([128, L], fp16, name="obh", tag="obh")
            nc.scalar.activation(obh[:], ps_prev[:, L * half : L * (half + 1)],
                                 AF.Identity, bias=f3b_sb)
            ncols = L // len(engs)
            for c, eng in enumerate(engs):
                lo = ncols * c
                eng.dma_start(
                    out=out[g_prev][:, L * half + lo : L * half + lo + ncols],
                    in_=obh[:, lo : lo + ncols])

        pair_ps = {}
        deferred = []  # (ps, o) waiting for key
        for g in range(PAIRS):
            # one-behind flush (p0 and p1 wait for the deferred out1s)
            if g == 2:
                flush(pair_ps[0], 0)
                flush(pair_ps[1], 1)
            elif g >= 3:
                flush(pair_ps[g - 1], g - 1)
            ps = mp.tile([128, 2 * L], fp32, name="ps")
            pair_ps[g] = ps
            for oo in range(2):
                o = 2 * g + oo
                hs = emit_h(o)
                if o < DEFER:
                    emit_out2(ps, o, hs, first=True, last=False)
                    deferred.append((ps, o))
                else:
                    emit_out1(ps, o, first=True)
                    emit_out2(ps, o, hs, first=False)
                if o == 0:
                    emit_key_cast(psK0, 0, 0)
                if o == 1:
                    emit_key_cast(psK0, 0, 1)
                    psK1 = emit_key_mms(1)
                if o == 2:
                    emit_key_cast(psK1, 1, 0)
                    emit_key_cast(psK1, 1, 1)
                if o == DEFER - 1:
                    for dps, do in deferred:
                        emit_out1(dps, do, first=False, last=True)
                    deferred = []
        for dps, do in deferred:  # DEFER > OCTS edge case
            emit_out1(dps, do, first=False, last=True)
        # tail: last pair in halves -- octet-14's half overlaps octet-15's
        # out matmuls; octet-15's DMA splits across the idle queues
        flush_half(pair_ps[PAIRS - 1], PAIRS - 1, 0, [nc.sync])
        flush_half(pair_ps[PAIRS - 1], PAIRS - 1, 1, [nc.scalar, nc.gpsimd])

    nc.compile()
    return nc


def _get_nc():
    if "nc" not in _cache:
        _cache["nc"] = build_nc()
    return _cache["nc"]


def _chunk_major(a, nchunks):
    # [128*nchunks, W] -> [128, nchunks*W] with chunk-major free layout
    W = a.shape[1]
    return np.ascontiguousarray(
        a.reshape(nchunks, 128, W).transpose(1, 0, 2).reshape(128, nchunks * W))


def _make_in_maps(inputs):
    x = np.asarray(inputs["x"], np.float32)
    f32 = lambda a: np.asarray(a, np.float32)
    f16 = np.float16

    f2W = f32(inputs["f2W"])
    f3WT = f32(inputs["f3W"]).T  # [E, N]
    f3pad = np.zeros((E, 48), np.float32)
    for s in range(2):
        f3pad[:, 24 * s + 12 * s : 24 * s + 12 * s + N] = f3WT

    misc = np.zeros((128, MISC_W), np.float32)
    o_ = 0
    misc[:, o_ : o_ + 2] = f32(inputs["sb"]).reshape(EC, 128).T; o_ += 2
    misc[:, o_ : o_ + 2] = f32(inputs["tb"]).reshape(EC, 128).T; o_ += 2
    misc[:, o_ : o_ + 2] = f32(inputs["f2b"]).reshape(EC, 128).T; o_ += 2
    for k in range(4):
        for s in range(2):
            misc[32 * k + 12 * s : 32 * k + 12 * s + N, o_] = f32(inputs["f3b"])
    o_ += 1

    # small = [WvT | WuT | f3pad], all chunk-major fp16 (Wv ships first)
    small = np.concatenate([
        _chunk_major(f2W[:, E:].T, EC),
        _chunk_major(f2W[:, :E].T, EC),
        _chunk_major(f3pad, EC),
    ], axis=1).astype(f16)

    # blW blocks b = 2*ec + epc: within epc-major chunk layout, the ec slice
    # of the (e_out n) axis is the contiguous cols [1536*ec, 1536*(ec+1))
    blW_cm = _chunk_major(f32(inputs["blW"]).T, EC)  # [128, epc*3072 + (e n)]
    BLK = E * N // 2
    blocks = []
    for ec in range(EC):
        for epc in range(EC):
            blocks.append(blW_cm[:, 2 * BLK * epc + BLK * ec :
                                 2 * BLK * epc + BLK * (ec + 1)])
    blWm = np.ascontiguousarray(np.concatenate(blocks, axis=1)).astype(f16)

    shared = {
        "sWTm": _chunk_major(f32(inputs["sW"]).T, HC).astype(f16),
        "tWTm": _chunk_major(f32(inputs["tW"]).T, HC).astype(f16),
        "smallm": small,
        "blWm": blWm,
        "misc": misc,
    }

    in_maps = []
    for c in range(NCORES):
        b, r = divmod(c, 4)
        m = dict(shared)
        m["xTm"] = _chunk_major(np.ascontiguousarray(x[b].T), HC).astype(f16)
        m["xTim"] = _chunk_major(
            np.ascontiguousarray(x[b, IB * r : IB * (r + 1), :].T), HC).astype(f16)
        in_maps.append(m)
    return in_maps


def _gather(results):
    full = np.empty((B, L, N, L), np.float32)
    for c in range(NCORES):
        b, r = divmod(c, 4)
        raw = results[c]["out"]  # [PAIRS, 128, 2L] fp16
        # row 32k + 12s + n of pair g, col 512*o + j  ->  (i = 16g+8o+2k+s, n, j)
        v = raw.reshape(PAIRS, 4, 32, 2, L)[:, :, :24]  # drop pads
        v = v.reshape(PAIRS, 4, 2, N, 2, L)  # [g, k, s, n, o, j]
        v = v.transpose(0, 4, 1, 2, 3, 5)  # [g, o, k, s, n, j]
        full[b, IB * r : IB * (r + 1)] = v.reshape(IB, N, L)
    return full


def kernel(x, sW, sb, tW, tb, f2W, f2b, f3W, f3b, blW):
    from concourse.bass_utils import run_bass_kernel_spmd

    in_maps = _make_in_maps(dict(
        x=x, sW=sW, sb=sb, tW=tW, tb=tb, f2W=f2W, f2b=f2b,
        f3W=f3W, f3b=f3b, blW=blW,
    ))
    nc = _get_nc()
    res = run_bass_kernel_spmd(nc, in_maps, core_ids=list(range(NCORES)))
    return _gather(res.results)
